# revision 20
# baseline (speedup 1.0000x reference)
"""Bahdanau attention (B=64, S=2048, H=512) on 8 Trainium2 NeuronCores.

Data-parallel: batch dim sharded 8 ways (8 batches/core), no collectives.

The Bass program is specialized on the per-batch valid-tile counts derived
from src_lengths (tokens past a batch's length only contribute exact zeros
to the outputs, so their score/context tiles are skipped entirely).  To keep
one SPMD program across the 8 cores, batches are sorted by tile count and
dealt round-robin: core c, slot j gets the (8*j+c)-th largest batch, and
slot j's tile count is the max of its group (= the (8*j)-th largest).
Host permutes inputs / un-permutes outputs.

Algorithm per core (per batch slot, S tiled into <=16 x 128 tokens):
  scores:   psum[tok,512]  = ones.T @ qs[b]          (K=1 f32r matmul: +qs)
                           += sum_k encT_k.T @ W_h_k (bf16 matmuls)
            tanh -> bf16 SBUF                        (ACT)
            scores[:,t] = sum_h tanh*v_rep + mask    (fused DVE TTR)
  softmax:  p = exp(scores) with fused rowsum        (ACT)
            l = partition-sum(rowsum), 1/l, broadcast via ones-matmuls (PE)
            attn = p/l (ACT) -> PE transpose -> DMA out
  context:  psum[1,512] = sum_t p_bf16[:,t].T @ enc_n[t]  (bf16 matmuls)
  final:    cat=[ctx*1/l, q] -> PE transpose -> cat.T @ W_out -> tanh -> DMA

Host side: computes qs = query@W_s, ships encoder outputs twice (transposed
bf16 [B,H,S] for the scores matmul, natural bf16 [B,S,H] for context),
replicated v, additive length mask, then gathers per-core outputs.
"""
import os
import sys

import numpy as np
import ml_dtypes

for _p in ("/opt/trn_rl_repo",):
    if _p not in sys.path:
        sys.path.insert(0, _p)

from contextlib import ExitStack

import concourse.bass as bass
import concourse.tile as tile
from concourse import mybir, bass_isa
from concourse.bass_utils import run_bass_kernel_spmd

B, S, H = 64, 2048, 512
NCORES = 8
BPC = B // NCORES      # 8 batch slots per core
NT = S // 128          # 16 token tiles per full batch
KT = H // 128          # 4 contraction tiles
F32 = mybir.dt.float32
F32R = mybir.dt.float32r
BF16 = mybir.dt.bfloat16
AF = mybir.ActivationFunctionType
ALU = mybir.AluOpType

_CACHE = {}

# this container's walrus codegen accepts only one sync-wait command per
# instruction; hoist extras onto standalone NoOps on the same engine
_MAX_WAITS = 1


def _split_waits(nc, max_waits=_MAX_WAITS):
    n_new = 0
    for bb in nc.main_func.blocks:
        insts = bb.instructions
        i = 0
        while i < len(insts):
            ins = insts[i]
            si = ins.sync_info
            if si is not None and si.on_wait and len(si.on_wait) > max_waits:
                waits = list(si.on_wait)
                keep = waits[:max_waits]
                extra = waits[max_waits:]
                new_insts = []
                for j in range(0, len(extra), max_waits):
                    chunk = extra[j:j + max_waits]
                    nop = mybir.InstNoOp(
                        name=f"{ins.name}-wsplit{j}", ins=[], outs=[]
                    )
                    nop.engine = ins.engine
                    nop.sync_info = mybir.SyncInfo(on_wait=chunk, on_update=[])
                    new_insts.append(nop)
                ins.sync_info = mybir.SyncInfo(
                    on_wait=keep, on_update=si.on_update
                )
                for k, nop in enumerate(new_insts):
                    insts.insert(i + k, nop)
                n_new += len(new_insts)
                i += len(new_insts)
            i += 1
    return n_new


def _build_bass(slot_counts, split=True):
    """slot_counts[j] = number of 128-token tiles computed for batch slot j."""
    nc = bass.Bass()
    enc_t = nc.declare_dram_parameter("enc_t", [BPC, H, S], BF16, isOutput=False)
    enc_n = nc.declare_dram_parameter("enc_n", [BPC, S, H], BF16, isOutput=False)
    qs_lin = nc.declare_dram_parameter("qs_lin", [1, BPC * H + 128], F32R, isOutput=False)
    maskp = nc.declare_dram_parameter("maskp", [BPC, 128, NT], F32, isOutput=False)
    v_rep = nc.declare_dram_parameter("v_rep", [128, H], BF16, isOutput=False)
    w_h = nc.declare_dram_parameter("w_h", [H, H], BF16, isOutput=False)
    w_out = nc.declare_dram_parameter("w_out", [2 * H, H], F32, isOutput=False)
    query = nc.declare_dram_parameter("query", [BPC, H], F32, isOutput=False)
    ident = nc.declare_dram_parameter("ident", [128, 128], F32, isOutput=False)
    wo_o = nc.declare_dram_parameter("wo_o", [BPC, H], F32, isOutput=True)
    attn_o = nc.declare_dram_parameter("attn_o", [BPC, S], F32, isOutput=True)

    with tile.TileContext(nc) as tc, ExitStack() as ctx:
        singles = ctx.enter_context(tc.tile_pool(name="singles", bufs=1))
        big = ctx.enter_context(tc.tile_pool(name="big", bufs=2))
        work = ctx.enter_context(tc.tile_pool(name="work", bufs=3))
        sm = ctx.enter_context(tc.tile_pool(name="sm", bufs=2))
        pes = ctx.enter_context(tc.tile_pool(name="pes", bufs=3, space="PSUM"))
        pctx = ctx.enter_context(tc.tile_pool(name="pctx", bufs=2, space="PSUM"))
        psm = ctx.enter_context(tc.tile_pool(name="psm", bufs=2, space="PSUM"))

        wh_sb = singles.tile([128, KT, H], BF16)
        nc.sync.dma_start(out=wh_sb, in_=w_h[:].rearrange("(k p) n -> p k n", p=128))
        qsl_sb = singles.tile([1, BPC * H + 128], F32R)
        nc.sync.dma_start(out=qsl_sb, in_=qs_lin[:])
        onesr_sb = qsl_sb[0:1, BPC * H:BPC * H + 128]
        wo_sb = singles.tile([128, 2 * KT, H], F32)
        nc.sync.dma_start(out=wo_sb, in_=w_out[:].rearrange("(k p) n -> p k n", p=128))
        vr_sb = singles.tile([128, H], BF16)
        nc.sync.dma_start(out=vr_sb, in_=v_rep[:])
        id_sb = singles.tile([128, 128], F32)
        nc.sync.dma_start(out=id_sb, in_=ident[:])
        ones_sb = singles.tile([128, 128], F32)
        nc.vector.memset(ones_sb, 1.0)
        ctxall = singles.tile([1, BPC, H], F32)
        # PE pre-touch of preloaded tiles so later PE instructions carry at
        # most one new semaphore wait (self-loading matmuls allow only one)
        dummy_ps = psm.tile([128, 128], F32, tag="small")
        nc.tensor.matmul(dummy_ps, lhsT=id_sb, rhs=id_sb)
        dummy_ps2 = psm.tile([128, 128], F32, tag="small")
        nc.tensor.matmul(dummy_ps2, lhsT=wo_sb[:, 0, 0:128], rhs=wo_sb[:, 0, 0:128])

        for b in range(BPC):
            C = slot_counts[b]
            if C == 0:
                nc.vector.memset(ctxall[:, b, :], 0.0)
                continue
            et = big.tile([128, KT, C * 128], BF16, tag="encT")
            nc.sync.dma_start(
                out=et,
                in_=enc_t[b][:, 0:C * 128].rearrange("(k p) s -> p k s", p=128),
            )
            en = big.tile([128, C, H], BF16, tag="encN")
            nc.sync.dma_start(
                out=en,
                in_=enc_n[b][0:C * 128].rearrange("(t p) h -> p t h", p=128),
            )
            mk = work.tile([128, C], F32, tag="mask")
            nc.sync.dma_start(out=mk, in_=maskp[b][:, 0:C])

            scores = sm.tile([128, C], F32, tag="scores")
            for t in range(C):
                pe = pes.tile([128, H], F32, tag="es")
                # first matmul broadcasts qs[b] into every token row (K=1)
                nc.tensor.matmul(
                    pe,
                    lhsT=onesr_sb,
                    rhs=qsl_sb[0:1, b * H:(b + 1) * H],
                    start=True,
                    stop=False,
                )
                for k in range(KT):
                    nc.tensor.matmul(
                        pe,
                        lhsT=et[:, k, 128 * t:128 * (t + 1)],
                        rhs=wh_sb[:, k, :],
                        start=False,
                        stop=(k == KT - 1),
                    )
                th = work.tile([128, H], BF16, tag="tanh")
                nc.scalar.activation(out=th, in_=pe, func=AF.Tanh)
                waste = work.tile([128, H], BF16, tag="waste")
                nc.vector.tensor_mul(out=waste, in0=th, in1=vr_sb)
                nc.vector.tensor_reduce(
                    out=scores[:, t:t + 1],
                    in_=waste,
                    axis=mybir.AxisListType.X,
                    op=ALU.add,
                )

            nc.vector.tensor_add(out=scores, in0=scores, in1=mk)
            pf = sm.tile([128, C], F32, tag="pf")
            rowsum = sm.tile([128, 1], F32, tag="rowsum")
            nc.scalar.activation(out=pf, in_=scores, func=AF.Exp, accum_out=rowsum)
            pb = sm.tile([128, C], BF16, tag="pb")
            nc.vector.tensor_copy(out=pb, in_=pf)

            rowsum2 = sm.tile([128, 1], F32, tag="rowsum2")
            nc.vector.tensor_copy(out=rowsum2, in_=rowsum)
            # l replicated on every partition: ones128.T @ rowsum  (K=128 f32)
            lrep = psm.tile([128, 1], F32, tag="small")
            nc.tensor.matmul(lrep, lhsT=ones_sb, rhs=rowsum2)
            rlr = sm.tile([128, 1], F32, tag="rlrep")
            nc.vector.reciprocal(out=rlr, in_=lrep)

            at = sm.tile([128, C], F32, tag="attn")
            nc.vector.tensor_scalar_mul(out=at, in0=pf, scalar1=rlr)
            pat = psm.tile([C, 128], F32, tag="small")
            nc.tensor.transpose(pat, at, id_sb)
            atT = sm.tile([C, 128], F32, tag="attnT")
            nc.vector.tensor_copy(out=atT, in_=pat)
            nc.sync.dma_start(
                out=attn_o[b].rearrange("(t q) -> t q", q=128)[0:C], in_=atT
            )

            en_touch = psm.tile([128, 128], F32, tag="small")
            nc.tensor.matmul(
                en_touch, lhsT=en[:, 0, 0:128], rhs=en[:, 0, 128:256]
            )
            pc = pctx.tile([1, H], F32, tag="ctx")
            for t in range(C):
                nc.tensor.matmul(
                    pc,
                    lhsT=pb[:, t:t + 1],
                    rhs=en[:, t, :],
                    start=(t == 0),
                    stop=(t == C - 1),
                )
            nc.vector.tensor_scalar_mul(
                out=ctxall[:, b, :], in0=pc, scalar1=rlr[0:1, :]
            )

        # final projection: w_out = tanh([ctx, q] @ W_out) per batch
        ctxc = singles.tile([BPC, H], F32)
        nc.sync.dma_start(out=ctxc, in_=ctxall)
        qc = singles.tile([BPC, H], F32)
        nc.sync.dma_start(out=qc, in_=query[:])
        catT = singles.tile([128, 2 * KT, BPC], F32)
        for j in range(2 * KT):
            half, jj = (ctxc, j) if j < KT else (qc, j - KT)
            pcT = psm.tile([128, BPC], F32, tag="small")
            nc.tensor.transpose(
                pcT, half[:, 128 * jj:128 * (jj + 1)], id_sb[0:BPC, 0:BPC]
            )
            nc.vector.tensor_copy(out=catT[:, j, :], in_=pcT)
        pw = psm.tile([BPC, H], F32, tag="small")
        for j in range(2 * KT):
            nc.tensor.matmul(
                pw,
                lhsT=catT[:, j, :],
                rhs=wo_sb[:, j, :],
                start=(j == 0),
                stop=(j == 2 * KT - 1),
            )
        wo_sbuf = singles.tile([BPC, H], F32)
        nc.scalar.activation(out=wo_sbuf, in_=pw, func=AF.Tanh)
        nc.sync.dma_start(out=wo_o[:], in_=wo_sbuf)
    if split:
        _split_waits(nc)
    return nc


def _get_nc(slot_counts):
    key = tuple(slot_counts)
    if key not in _CACHE:
        _CACHE[key] = _build_bass(key)
    return _CACHE[key]


def _prep(query, encoder_outputs, src_lengths, W_h, W_s, v, W_out):
    query = np.ascontiguousarray(np.asarray(query, np.float32))
    enc = np.ascontiguousarray(np.asarray(encoder_outputs, np.float32))
    lens = np.asarray(src_lengths).astype(np.int64)
    W_h = np.ascontiguousarray(np.asarray(W_h, np.float32))
    W_s = np.ascontiguousarray(np.asarray(W_s, np.float32))
    v = np.asarray(v, np.float32)
    W_out = np.ascontiguousarray(np.asarray(W_out, np.float32))

    # batch -> (core, slot) assignment: sort by tile count, deal round-robin
    nt = np.ceil(lens / 128).astype(int)          # valid tiles per batch
    order = np.argsort(-nt, kind="stable")        # batch ids, largest first
    slot_counts = tuple(int(nt[order[8 * j]]) for j in range(BPC))

    qs = (query @ W_s).astype(np.float32)
    enc_bf = enc.astype(ml_dtypes.bfloat16)
    enc_t = np.ascontiguousarray(enc_bf.transpose(0, 2, 1))    # (B, H, S)
    enc_n = enc_bf                                             # (B, S, H)
    v_rep = np.broadcast_to(
        v.astype(ml_dtypes.bfloat16)[None, :], (128, H)
    ).copy()
    W_h_bf = W_h.astype(ml_dtypes.bfloat16)
    tok = np.arange(S).reshape(NT, 128).T                      # [128, NT]
    mask = np.where(tok[None, :, :] < lens[:, None, None], 0.0, -1e9).astype(
        np.float32
    )
    ident = np.eye(128, dtype=np.float32)

    in_maps = []
    for c in range(NCORES):
        ids = [int(order[8 * j + c]) for j in range(BPC)]
        in_maps.append(
            {
                "enc_t": np.ascontiguousarray(enc_t[ids]),
                "enc_n": np.ascontiguousarray(enc_n[ids]),
                "qs_lin": np.concatenate(
                    [qs[ids].reshape(-1), np.ones(128, np.float32)]
                ).reshape(1, BPC * H + 128),
                "maskp": np.ascontiguousarray(mask[ids]),
                "v_rep": v_rep,
                "w_h": W_h_bf,
                "w_out": W_out,
                "query": np.ascontiguousarray(query[ids]),
                "ident": ident,
            }
        )
    return in_maps, order, slot_counts


def kernel(query, encoder_outputs, src_lengths, W_h, W_s, v, W_out,
           _trace=False, _trace_kwargs=None):
    in_maps, order, slot_counts = _prep(
        query, encoder_outputs, src_lengths, W_h, W_s, v, W_out
    )
    nc = _get_nc(slot_counts)
    res = run_bass_kernel_spmd(
        nc,
        in_maps,
        core_ids=list(range(NCORES)),
        trace=_trace,
        **(_trace_kwargs or {}),
    )
    w_out_full = np.zeros((B, H), np.float32)
    attn_full = np.zeros((B, S), np.float32)
    for c in range(NCORES):
        wo = np.asarray(res.results[c]["wo_o"])
        at = np.asarray(res.results[c]["attn_o"])
        for j in range(BPC):
            bid = int(order[8 * j + c])
            w_out_full[bid] = wo[j]
            attn_full[bid] = at[j]
    if _trace:
        kernel._last_result = res
    return (
        w_out_full.reshape(B, 1, H).astype(np.float32),
        attn_full.reshape(B, S, 1).astype(np.float32),
    )
